# revision 2
# baseline (speedup 1.0000x reference)
"""Multi-head causal self-attention on 8 Trainium2 NeuronCores.

Problem: B=2, S=2048, E=1024, H=16 heads (D=64), causal mask, f32 I/O.

Sharding: (batch x head-group) -> 8 cores. Core c handles batch b=c//4 and
4 heads h0=4*(c%4).. (column-parallel Q/K/V projections, local attention,
row-parallel partial output projection). The 4 partial outputs per batch are
summed on the host (the "all-reduce" of row-parallel TP), where the output
bias bo and the folded V-bias term (bv @ Wo.T, exact because softmax rows
sum to 1) are also added. Partial outputs travel as bf16 (summed in f32 on
the host); the rel-error budget absorbs the rounding.

Device kernel layout (all matmuls bf16 with f32 PSUM accumulate):
  - Host pre-transposes activations/weights so the kernel never transposes:
      qhT/khT = Wq_h @ q[b].T  (projection emits [d, s] directly)
  - qhT/khT live as [128, 2, S] pair-slots: head 2m in partitions 0:64 of
    slot m, head 2m+1 in partitions 64:128. The QK matmuls slice the real
    64-partition halves, so the two heads of a pair run CONCURRENTLY as
    row-tiled (tile_position (0,0) / (64,0)) matmuls on the two halves of
    the PE array - 2x QK throughput, and no zero-padding memsets.
  - scores^T [k, q] for the head pair land in one 2-bank PSUM tile
    [128, 2, 512]; ONE ScalarE exp covers both heads (the attention phase
    is exp-throughput-bound, so ACT instruction count matters).
  - attn^T feeds AV as the moving operand:
      ctx^T [d, q] = matmul(lhsT=V_aug [k, 128], rhs=attn^T)
    where V_aug cols 64:128 are ones, so rows 64:127 of the AV psum are the
    softmax row-sums pre-broadcast across 64 partitions (DVE cannot
    broadcast along partitions). Normalization is a DVE reciprocal+mul.
  - 1/sqrt(D) is folded into Wq/bq on the host.
  - Causal structure is exploited: only lower-triangular k-blocks are
    computed; the 128-wide diagonal band is masked by a multiplicative
    [128, 2, 128] triu tile after exp (exact: exp(s)*0 == 0).
"""

import os
import sys

for _p in ("/opt/trn_rl_repo",):
    if _p not in sys.path and os.path.isdir(_p):
        sys.path.insert(0, _p)

import numpy as np
import ml_dtypes

import concourse.bacc as bacc
from concourse import mybir
from concourse.tile import TileContext
from concourse.bass_utils import run_bass_kernel_spmd

BF16 = ml_dtypes.bfloat16
P = 128
B, S, E, H, D = 2, 2048, 1024, 16, 64
HPC = 4            # heads per core
DC = HPC * D       # 256 output dims per core per projection
NCORES = 8
QSUP = 512         # q-superblock (matmul free dim)
NSUP = S // QSUP   # 4
NKB = S // P       # 16 k-blocks
SCALE = float(np.sqrt(D))

AF = mybir.ActivationFunctionType
f32 = mybir.dt.float32
bf16 = mybir.dt.bfloat16

_CACHE = {}
LAST = {}


def _install_axon_profile_shim():
    """Provide antenv.axon_hooks (absent in this image) so
    run_bass_kernel_spmd(trace=True) can NTFF-profile via libaxon_pjrt.so."""
    try:
        import antenv.axon_hooks  # noqa: F401
        return
    except ImportError:
        pass
    import contextlib
    import ctypes
    import types

    import antenv

    state = {"hook": None, "tried": False}

    def _build_hook():
        so_path = "/opt/axon/libaxon_pjrt.so"
        if not os.path.exists(so_path):
            return None
        lib = ctypes.CDLL(so_path)
        if not hasattr(lib, "axon_start_nrt_profile"):
            return None
        lib.axon_start_nrt_profile.argtypes = [
            ctypes.POINTER(ctypes.c_int64),
            ctypes.c_size_t,
        ]
        lib.axon_start_nrt_profile.restype = ctypes.c_int64
        lib.axon_stop_nrt_profile.argtypes = [ctypes.c_char_p]
        lib.axon_stop_nrt_profile.restype = ctypes.c_int64

        @contextlib.contextmanager
        def _hook(output_dir, device_ids):
            import jax

            jax.devices()
            if device_ids:
                ids = (ctypes.c_int64 * len(device_ids))(*device_ids)
                rc = lib.axon_start_nrt_profile(ids, len(device_ids))
            else:
                rc = lib.axon_start_nrt_profile(None, 0)
            if rc != 0:
                raise RuntimeError(f"axon_start_nrt_profile rc={rc}")
            try:
                yield
            finally:
                n = lib.axon_stop_nrt_profile(str(output_dir).encode())
                if n < 0:
                    raise RuntimeError(f"axon_stop_nrt_profile rc={n}")
                print(f"profile: {n} file(s) written to {output_dir}")

        return _hook

    mod = types.ModuleType("antenv.axon_hooks")

    def set_axon_ntff_profile_hook(h):
        state["hook"] = h
        state["tried"] = True

    def get_axon_ntff_profile_hook():
        if not state["tried"]:
            state["hook"] = _build_hook()
            state["tried"] = True
        return state["hook"]

    mod.set_axon_ntff_profile_hook = set_axon_ntff_profile_hook
    mod.get_axon_ntff_profile_hook = get_axon_ntff_profile_hook
    sys.modules["antenv.axon_hooks"] = mod
    antenv.axon_hooks = mod


_install_axon_profile_shim()


def _build_nc(causal: bool):
    nc = bacc.Bacc(None, target_bir_lowering=False)

    xqT = nc.dram_tensor("xqT", [E, S], bf16, kind="ExternalInput")
    xkT = nc.dram_tensor("xkT", [E, S], bf16, kind="ExternalInput")
    xvT = nc.dram_tensor("xvT", [E, S], bf16, kind="ExternalInput")
    wqT = nc.dram_tensor("wqT", [P, 8, DC], bf16, kind="ExternalInput")
    wkT = nc.dram_tensor("wkT", [P, 8, DC], bf16, kind="ExternalInput")
    wvT = nc.dram_tensor("wvT", [P, 8, DC], bf16, kind="ExternalInput")
    woT = nc.dram_tensor("woT", [P, 2, E], bf16, kind="ExternalInput")
    bqk = nc.dram_tensor("bqk", [P, 4], f32, kind="ExternalInput")
    cmask = nc.dram_tensor("cmask", [P, 2, P], bf16, kind="ExternalInput")
    out = nc.dram_tensor("out", [S, E], bf16, kind="ExternalOutput")

    with TileContext(nc) as tc:
        with (
            tc.tile_pool(name="consts", bufs=1) as consts,
            tc.tile_pool(name="xin", bufs=16) as xin,
            tc.tile_pool(name="acts", bufs=1) as acts,
            tc.tile_pool(name="attn", bufs=3) as attn,
            tc.tile_pool(name="norm", bufs=2) as norm,
            tc.tile_pool(name="osb", bufs=3) as osb,
            tc.tile_pool(name="ppool", bufs=2, space="PSUM") as ppool,
            tc.tile_pool(name="stp", bufs=2, space="PSUM") as stp,
            tc.tile_pool(name="cpool", bufs=2, space="PSUM") as cpool,
        ):
            # ---- HAM warm-up -----------------------------------------------
            # A dependency-free burst of matmuls on a zeroed scratch tile
            # warms the PE clock gate (~3.4us of activity needed) while the
            # first input DMAs stream (results are never read).
            warm = consts.tile([P, QSUP], bf16)
            nc.vector.memset(warm[:], 0.0)
            for wi in range(8):
                wp = ppool.tile([P, QSUP], f32, tag="ps", name=f"warm_{wi}")
                nc.tensor.matmul(wp, warm[:, 0:P], warm[:], start=True,
                                 stop=True)

            # ---- constants -------------------------------------------------
            wq_sb = consts.tile([P, 8, DC], bf16)
            wk_sb = consts.tile([P, 8, DC], bf16)
            wv_sb = consts.tile([P, 8, DC], bf16)
            wo_sb = consts.tile([P, 2, E], bf16)
            nc.sync.dma_start(wq_sb, wqT[:])
            bqk_sb = consts.tile([P, 4], f32)
            nc.sync.dma_start(bqk_sb[:], bqk[:])
            if causal:
                cm_sb = consts.tile([P, 2, P], bf16)
                nc.sync.dma_start(cm_sb[:], cmask[:])

            # ---- activations ----------------------------------------------
            # qhT/khT pair-slots: [128, m, S]; head 2m in partitions 0:64,
            # head 2m+1 in partitions 64:128 (matches the bias layout, so
            # one full-width tensor_scalar_add drains each PSUM chain).
            qhT = acts.tile([P, 2, S], bf16)
            khT = acts.tile([P, 2, S], bf16)
            # V natural layout + ones block: [:, sb, h, 0:64] = vh, 64:128 ones
            vha = acts.tile([P, NKB, HPC, 2 * D], bf16)
            ctxT = acts.tile([P, 2, S], bf16)
            nc.vector.memset(vha[:, :, :, D:], 1.0)

            # ---- Q/K/V projections ----------------------------------------
            def load_x(xT):
                xr = xT.rearrange("(ko p) s -> ko p s", p=P)
                tiles = []
                for ko in range(8):
                    t = xin.tile([P, S], bf16, tag="xin")
                    nc.sync.dma_start(t, xr[ko])
                    tiles.append(t)
                return tiles

            for pj, (xT, w_sb, bcol, dst) in enumerate(
                ((xqT, wq_sb, 0, qhT), (xkT, wk_sb, 2, khT))
            ):
                if pj == 1:
                    nc.sync.dma_start(wk_sb, wkT[:])
                xt = load_x(xT)
                for m in range(2):
                    # ns pairs: 2 live accumulation chains; ko outer so the
                    # stationary w tile is reused by 2 consecutive matmuls
                    # and the first matmul only needs x tile 0 DMA'd.
                    for np_ in range(2):
                        chains = [
                            ppool.tile([P, QSUP], f32, tag="ps",
                                       name=f"pj_{pj}_{m}_{np_}_{nh}")
                            for nh in range(2)
                        ]
                        for ko in range(8):
                            for nh in range(2):
                                ns = 2 * np_ + nh
                                nc.tensor.matmul(
                                    chains[nh],
                                    w_sb[:, ko, m * P:(m + 1) * P],
                                    xt[ko][:, ns * QSUP:(ns + 1) * QSUP],
                                    start=(ko == 0),
                                    stop=(ko == 7),
                                )
                        for nh in range(2):
                            ns = 2 * np_ + nh
                            nsl = slice(ns * QSUP, (ns + 1) * QSUP)
                            nc.vector.tensor_scalar_add(
                                dst[:, m, nsl], chains[nh],
                                bqk_sb[:, bcol + m:bcol + m + 1],
                            )

            nc.sync.dma_start(wv_sb, wvT[:])
            nc.sync.dma_start(wo_sb, woT[:])
            xt = load_x(xvT)
            for sb in range(NKB):
                ps = ppool.tile([P, DC], f32, tag="ps", name=f"vps_{sb}")
                for ko in range(8):
                    nc.tensor.matmul(
                        ps,
                        xt[ko][:, sb * P:(sb + 1) * P],
                        wv_sb[:, ko, :],
                        start=(ko == 0),
                        stop=(ko == 7),
                    )
                nc.vector.tensor_copy(
                    vha[:, sb, :, 0:D],
                    ps.rearrange("p (h d) -> p h d", h=HPC),
                )

            # ---- attention -------------------------------------------------
            for qs in range(NSUP):
                for m in range(2):        # head pair (local heads 2m, 2m+1)
                    nkb = 4 * qs + 4 if causal else NKB
                    cps = [
                        cpool.tile([P, QSUP], f32, tag="cps",
                                   name=f"cps_{m}_{qs}_{h2}")
                        for h2 in range(2)
                    ]
                    for kb in range(nkb):
                        r = kb - 4 * qs  # >=0 only inside the diagonal band
                        qlo = r * P if (causal and r >= 0) else 0
                        # scores for both heads of the pair: concurrent
                        # row-tiled matmuls into one 2-bank PSUM tile
                        st = stp.tile([P, 2, QSUP], f32, tag="st2",
                                      name=f"st_{m}_{qs}_{kb}")
                        nc.tensor.matmul(
                            st[:, 0, qlo:],
                            khT[0:D, m, kb * P:(kb + 1) * P],
                            qhT[0:D, m, qs * QSUP + qlo:(qs + 1) * QSUP],
                            start=True, stop=True,
                        )
                        nc.tensor.matmul(
                            st[:, 1, qlo:],
                            khT[D:, m, kb * P:(kb + 1) * P],
                            qhT[D:, m, qs * QSUP + qlo:(qs + 1) * QSUP],
                            start=True, stop=True,
                        )
                        at = attn.tile([P, 2, QSUP], bf16, tag="at",
                                       name=f"at_{m}_{qs}_{kb}")
                        nc.scalar.activation(at[:, :, qlo:], st[:, :, qlo:],
                                             AF.Exp)
                        if causal and r >= 0:
                            nc.vector.tensor_mul(
                                at[:, :, qlo:qlo + P],
                                at[:, :, qlo:qlo + P],
                                cm_sb,
                            )
                        for h2 in range(2):
                            nc.tensor.matmul(
                                cps[h2][:, qlo:],
                                vha[:, kb, 2 * m + h2, :],
                                at[:, h2, qlo:],
                                start=(kb == 0), stop=(kb == nkb - 1),
                            )
                    # normalization: both heads' row-sums into one tile,
                    # one reciprocal, two muls into ctxT pair-slots
                    sums = norm.tile([P, QSUP], f32, tag="sums")
                    nc.vector.tensor_copy(out=sums[0:D, :], in_=cps[0][D:, :])
                    nc.vector.tensor_copy(out=sums[D:, :], in_=cps[1][D:, :])
                    rec = norm.tile([P, QSUP], f32, tag="rec")
                    nc.vector.reciprocal_approx_fast(out=rec, in_=sums)
                    qsl = slice(qs * QSUP, (qs + 1) * QSUP)
                    nc.vector.tensor_mul(
                        ctxT[0:D, m, qsl], cps[0][0:D, :], rec[0:D, :],
                    )
                    nc.vector.tensor_mul(
                        ctxT[D:, m, qsl], cps[1][0:D, :], rec[D:, :],
                    )

                # ---- output projection for this qs's 4 s-blocks ----------
                for sb in range(4 * qs, 4 * qs + 4):
                    pso = [
                        ppool.tile([P, QSUP], f32, tag="ps",
                                   name=f"pso_{sb}_{n2}")
                        for n2 in range(2)
                    ]
                    for km in range(2):
                        for n2 in range(2):
                            nc.tensor.matmul(
                                pso[n2],
                                ctxT[:, km, sb * P:(sb + 1) * P],
                                wo_sb[:, km, n2 * QSUP:(n2 + 1) * QSUP],
                                start=(km == 0), stop=(km == 1),
                            )
                    for n2 in range(2):
                        ot = osb.tile([P, QSUP], bf16, tag="ot",
                                      name=f"ot_{sb}_{n2}")
                        nc.vector.tensor_copy(out=ot, in_=pso[n2])
                        nc.sync.dma_start(
                            out[sb * P:(sb + 1) * P,
                                n2 * QSUP:(n2 + 1) * QSUP], ot,
                        )

    nc.finalize()
    return nc


def _get_nc(causal: bool):
    key = ("nc", causal)
    if key not in _CACHE:
        _CACHE[key] = _build_nc(causal)
    return _CACHE[key]


def _bf(a):
    return np.ascontiguousarray(a, dtype=np.float32).astype(BF16)


def _wperm(wT, nko):
    """[nko*128, M] -> [128, nko, M] so each SBUF partition's data is one
    contiguous run in DRAM (single DMA descriptor per partition)."""
    wT = np.asarray(wT, np.float32)
    m = wT.shape[1]
    return np.ascontiguousarray(
        wT.reshape(nko, P, m).transpose(1, 0, 2)).astype(BF16)


def kernel(q, k, v, mask, Wq, bq, Wk, bk, Wv, bv, Wo, bo):
    q = np.asarray(q, np.float32)
    k = np.asarray(k, np.float32)
    v = np.asarray(v, np.float32)
    mask = np.asarray(mask)
    Wq, bq = np.asarray(Wq, np.float32), np.asarray(bq, np.float32)
    Wk, bk = np.asarray(Wk, np.float32), np.asarray(bk, np.float32)
    Wv, bv = np.asarray(Wv, np.float32), np.asarray(bv, np.float32)
    Wo, bo = np.asarray(Wo, np.float32), np.asarray(bo, np.float32)

    m2 = mask.reshape(S, S) != 0
    if m2.all():
        causal = False
    else:
        tri = np.tril(np.ones((S, S), bool))
        assert (m2 == tri).all(), "only causal or all-ones masks supported"
        causal = True

    nc = _get_nc(causal)

    cm1 = np.asarray(
        np.arange(P)[:, None] <= np.arange(P)[None, :], np.float32
    ).astype(BF16)  # [k, q] keep-region of the diagonal 128-band
    cm = np.ascontiguousarray(np.stack([cm1, cm1], axis=1))  # [P, 2, P]

    xT = {}
    for b in range(B):
        xT[("q", b)] = _bf(q[b].T)
        xT[("k", b)] = _bf(k[b].T)
        xT[("v", b)] = _bf(v[b].T)

    in_maps = []
    for c in range(NCORES):
        b = c // 4
        rows = slice((c % 4) * DC, (c % 4) * DC + DC)
        bq_s = (bq[rows] / SCALE).reshape(2, P).T
        bk_s = bk[rows].reshape(2, P).T
        in_maps.append({
            "xqT": xT[("q", b)],
            "xkT": xT[("k", b)],
            "xvT": xT[("v", b)],
            "wqT": _wperm(Wq[rows].T / SCALE, 8),
            "wkT": _wperm(Wk[rows].T, 8),
            "wvT": _wperm(Wv[rows].T, 8),
            "woT": _wperm(Wo[:, rows].T, 2),
            "bqk": np.ascontiguousarray(
                np.concatenate([bq_s, bk_s], axis=1), np.float32),
            "cmask": cm,
        })

    res = run_bass_kernel_spmd(nc, in_maps, core_ids=list(range(NCORES)))
    LAST["exec_time_ns"] = res.exec_time_ns
    LAST["results"] = res

    host_bias = (bo + bv @ Wo.T).astype(np.float32)
    out = np.zeros((B, S, E), np.float32)
    for c in range(NCORES):
        out[c // 4] += res.results[c]["out"].astype(np.float32)
    out += host_bias
    return out


# revision 5
# speedup vs baseline: 1.1132x; 1.1132x over previous
"""Multi-head causal self-attention on 8 Trainium2 NeuronCores.

Problem: B=2, S=2048, E=1024, H=16 heads (D=64), causal mask, f32 I/O.

Sharding: (batch x head-group) -> 8 cores. Core c handles batch b=c//4 and
4 heads h0=4*(c%4).. (column-parallel Q/K/V projections, local attention,
row-parallel partial output projection). The 4 partial outputs per batch are
summed on the host (the "all-reduce" of row-parallel TP), where the output
bias bo and the folded V-bias term (bv @ Wo.T, exact because softmax rows
sum to 1) are also added. Partial outputs travel as bf16 (summed in f32 on
the host); the rel-error budget absorbs the rounding.

The attention phase is ScalarE-exp-throughput-bound (~75us of EXP), so the
program is software-pipelined so every other engine's work hides behind the
exp stream:
  - scores for a head pair land in one 2-bank PSUM tile [128, 2, 512];
    ONE ScalarE exp instruction covers both heads (ACT count matters).
  - The AV matmuls are emitted LAG=2 k-blocks behind the QK/exp front, so
    they never stall the PE queue waiting on an exp that hasn't run.
  - The causal-diagonal masking multiplies run on the otherwise-idle GpSimd
    engine (exact: exp(s)*0 == 0).
  - Each q-superblock's output projection + drain is spread into the next
    superblock's kb loop, keeping ScalarE fed during the PE-side work.

Device kernel layout (all matmuls bf16 with f32 PSUM accumulate):
  - Host pre-transposes activations/weights so the kernel never transposes:
      qhT/khT = Wq_h @ q[b].T  (projection emits [d, s] directly)
  - khT is per-head zero-padded: [128, 4, S]; head h occupies partitions
    (h%2)*64..+64 of slot h, the complementary half is zeros. qhT is packed
    as pair-slots [128, 2, S] (head 2m in partitions 0:64, head 2m+1 in
    64:128): the QK matmul contracts khT's zero half against the other
    head's q data, which contributes exactly zero. This keeps the Q-side
    PSUM drain a single full-width tensor_scalar_add.
  - attn^T feeds AV as the moving operand:
      ctx^T [d, q] = matmul(lhsT=V_aug [k, 128], rhs=attn^T)
    where V_aug cols 64:128 are ones, so rows 64:127 of the AV psum are the
    softmax row-sums pre-broadcast across 64 partitions (DVE cannot
    broadcast along partitions). Normalization is a DVE reciprocal+mul.
  - 1/sqrt(D) is folded into Wq/bq on the host.
  - Causal structure is exploited: only lower-triangular k-blocks are
    computed.
"""

import os
import sys

for _p in ("/opt/trn_rl_repo",):
    if _p not in sys.path and os.path.isdir(_p):
        sys.path.insert(0, _p)

import numpy as np
import ml_dtypes

import concourse.bacc as bacc
from concourse import mybir
from concourse.tile import TileContext
from concourse.bass_utils import run_bass_kernel_spmd

BF16 = ml_dtypes.bfloat16
P = 128
B, S, E, H, D = 2, 2048, 1024, 16, 64
HPC = 4            # heads per core
DC = HPC * D       # 256 output dims per core per projection
NCORES = 8
QSUP = 512         # q-superblock (matmul free dim)
NSUP = S // QSUP   # 4
NKB = S // P       # 16 k-blocks
LAG = 2            # AV runs LAG k-blocks behind the QK/exp front
SCALE = float(np.sqrt(D))

AF = mybir.ActivationFunctionType
f32 = mybir.dt.float32
bf16 = mybir.dt.bfloat16

_CACHE = {}
LAST = {}


def _install_axon_profile_shim():
    """Provide antenv.axon_hooks (absent in this image) so
    run_bass_kernel_spmd(trace=True) can NTFF-profile via libaxon_pjrt.so."""
    try:
        import antenv.axon_hooks  # noqa: F401
        return
    except ImportError:
        pass
    import contextlib
    import ctypes
    import types

    import antenv

    state = {"hook": None, "tried": False}

    def _build_hook():
        so_path = "/opt/axon/libaxon_pjrt.so"
        if not os.path.exists(so_path):
            return None
        lib = ctypes.CDLL(so_path)
        if not hasattr(lib, "axon_start_nrt_profile"):
            return None
        lib.axon_start_nrt_profile.argtypes = [
            ctypes.POINTER(ctypes.c_int64),
            ctypes.c_size_t,
        ]
        lib.axon_start_nrt_profile.restype = ctypes.c_int64
        lib.axon_stop_nrt_profile.argtypes = [ctypes.c_char_p]
        lib.axon_stop_nrt_profile.restype = ctypes.c_int64

        @contextlib.contextmanager
        def _hook(output_dir, device_ids):
            import jax

            jax.devices()
            if device_ids:
                ids = (ctypes.c_int64 * len(device_ids))(*device_ids)
                rc = lib.axon_start_nrt_profile(ids, len(device_ids))
            else:
                rc = lib.axon_start_nrt_profile(None, 0)
            if rc != 0:
                raise RuntimeError(f"axon_start_nrt_profile rc={rc}")
            try:
                yield
            finally:
                n = lib.axon_stop_nrt_profile(str(output_dir).encode())
                if n < 0:
                    raise RuntimeError(f"axon_stop_nrt_profile rc={n}")
                print(f"profile: {n} file(s) written to {output_dir}")

        return _hook

    mod = types.ModuleType("antenv.axon_hooks")

    def set_axon_ntff_profile_hook(h):
        state["hook"] = h
        state["tried"] = True

    def get_axon_ntff_profile_hook():
        if not state["tried"]:
            state["hook"] = _build_hook()
            state["tried"] = True
        return state["hook"]

    mod.set_axon_ntff_profile_hook = set_axon_ntff_profile_hook
    mod.get_axon_ntff_profile_hook = get_axon_ntff_profile_hook
    sys.modules["antenv.axon_hooks"] = mod
    antenv.axon_hooks = mod


_install_axon_profile_shim()


def _build_nc(causal: bool):
    nc = bacc.Bacc(None, target_bir_lowering=False)

    xqT = nc.dram_tensor("xqT", [E, S], bf16, kind="ExternalInput")
    xkT = nc.dram_tensor("xkT", [E, S], bf16, kind="ExternalInput")
    xvT = nc.dram_tensor("xvT", [E, S], bf16, kind="ExternalInput")
    wqT = nc.dram_tensor("wqT", [P, 8, DC], bf16, kind="ExternalInput")
    wkT = nc.dram_tensor("wkT", [P, 8, DC], bf16, kind="ExternalInput")
    wvT = nc.dram_tensor("wvT", [P, 8, DC], bf16, kind="ExternalInput")
    woT = nc.dram_tensor("woT", [P, 2, E], bf16, kind="ExternalInput")
    bqk = nc.dram_tensor("bqk", [P, 4], f32, kind="ExternalInput")
    cmask = nc.dram_tensor("cmask", [P, 2, P], bf16, kind="ExternalInput")
    out = nc.dram_tensor("out", [S, E], bf16, kind="ExternalOutput")

    with TileContext(nc) as tc:
        with (
            tc.tile_pool(name="consts", bufs=1) as consts,
            tc.tile_pool(name="xin", bufs=16) as xin,
            tc.tile_pool(name="acts", bufs=1) as acts,
            tc.tile_pool(name="attn", bufs=4) as attn,
            tc.tile_pool(name="norm", bufs=2) as norm,
            tc.tile_pool(name="osb", bufs=3) as osb,
            tc.tile_pool(name="ppool", bufs=2, space="PSUM") as ppool,
            tc.tile_pool(name="stp", bufs=2, space="PSUM") as stp,
            tc.tile_pool(name="cpool", bufs=2, space="PSUM") as cpool,
        ):
            # ---- HAM warm-up -----------------------------------------------
            # A dependency-free burst of matmuls on a zeroed scratch tile
            # warms the PE clock gate (~3.4us of activity needed) while the
            # first input DMAs stream (results are never read).
            warm = consts.tile([P, QSUP], bf16)
            nc.vector.memset(warm[:], 0.0)
            for wi in range(8):
                wp = ppool.tile([P, QSUP], f32, tag="ps", name=f"warm_{wi}")
                nc.tensor.matmul(wp, warm[:, 0:P], warm[:], start=True,
                                 stop=True)

            # ---- constants -------------------------------------------------
            wq_sb = consts.tile([P, 8, DC], bf16)
            wk_sb = consts.tile([P, 8, DC], bf16)
            wv_sb = consts.tile([P, 8, DC], bf16)
            wo_sb = consts.tile([P, 2, E], bf16)
            nc.sync.dma_start(wq_sb, wqT[:])
            bqk_sb = consts.tile([P, 4], f32)
            nc.sync.dma_start(bqk_sb[:], bqk[:])
            if causal:
                cm_sb = consts.tile([P, 2, P], bf16)
                nc.sync.dma_start(cm_sb[:], cmask[:])

            # ---- activations ----------------------------------------------
            # qhT pair-slots: [128, m, S]; head 2m in partitions 0:64, head
            # 2m+1 in 64:128 (matches the bias layout -> one full-width
            # tensor_scalar_add drains each Q PSUM chain).
            qhT = acts.tile([P, 2, S], bf16)
            # khT per-head zero-padded slots (the stationary side must be
            # zero-padded so the QK contraction kills the other head's data
            # in qhT). Pads are zeroed on the idle GpSimd engine; they are
            # only read ~50us later by the attention phase.
            khT = acts.tile([P, 4, S], bf16)
            for h in range(4):
                if h % 2 == 0:
                    nc.gpsimd.memset(khT[D:, h, :], 0.0)
                else:
                    nc.gpsimd.memset(khT[0:D, h, :], 0.0)
            # V natural layout + ones block: [:, sb, h, 0:64] = vh, 64:128 ones
            vha = acts.tile([P, NKB, HPC, 2 * D], bf16)
            ctxT = acts.tile([P, 2, S], bf16)
            nc.vector.memset(vha[:, :, :, D:], 1.0)

            # ---- Q/K/V projections ----------------------------------------
            def load_x(xT):
                xr = xT.rearrange("(ko p) s -> ko p s", p=P)
                tiles = []
                for ko in range(8):
                    t = xin.tile([P, S], bf16, tag="xin")
                    nc.sync.dma_start(t, xr[ko])
                    tiles.append(t)
                return tiles

            def chain_tile(i, nm, width=QSUP):
                # alternate the two 1-bank PSUM pools so a chain's WAR
                # predecessor is 2 chains back (hides the DVE drain latency)
                pool, tag = ((ppool, "ps"), (cpool, "cps"))[i % 2]
                return pool.tile([P, width], f32, tag=tag, name=nm)

            for pj, (xT, w_sb, bcol) in enumerate(
                ((xqT, wq_sb, 0), (xkT, wk_sb, 2))
            ):
                if pj == 1:
                    nc.sync.dma_start(wk_sb, wkT[:])
                xt = load_x(xT)
                for m in range(2):
                    # ns pairs: 2 live accumulation chains; ko outer so the
                    # stationary w tile is reused by 2 consecutive matmuls
                    # and the first matmul only needs x tile 0 DMA'd.
                    for np_ in range(2):
                        chains = [
                            chain_tile(nh, f"pj_{pj}_{m}_{np_}_{nh}")
                            for nh in range(2)
                        ]
                        for ko in range(8):
                            for nh in range(2):
                                ns = 2 * np_ + nh
                                nc.tensor.matmul(
                                    chains[nh],
                                    w_sb[:, ko, m * P:(m + 1) * P],
                                    xt[ko][:, ns * QSUP:(ns + 1) * QSUP],
                                    start=(ko == 0),
                                    stop=(ko == 7),
                                )
                        for nh in range(2):
                            ns = 2 * np_ + nh
                            nsl = slice(ns * QSUP, (ns + 1) * QSUP)
                            bsl = bqk_sb[:, bcol + m:bcol + m + 1]
                            if pj == 0:
                                nc.vector.tensor_scalar_add(
                                    qhT[:, m, nsl], chains[nh], bsl,
                                )
                            else:
                                nc.vector.tensor_scalar_add(
                                    khT[0:D, 2 * m, nsl], chains[nh][0:D],
                                    bsl[0:D],
                                )
                                nc.vector.tensor_scalar_add(
                                    khT[D:, 2 * m + 1, nsl], chains[nh][D:],
                                    bsl[D:],
                                )

            nc.sync.dma_start(wv_sb, wvT[:])
            nc.sync.dma_start(wo_sb, woT[:])
            xt = load_x(xvT)
            for sb in range(NKB):
                ps = chain_tile(sb, f"vps_{sb}", width=DC)
                for ko in range(8):
                    nc.tensor.matmul(
                        ps,
                        xt[ko][:, sb * P:(sb + 1) * P],
                        wv_sb[:, ko, :],
                        start=(ko == 0),
                        stop=(ko == 7),
                    )
                nc.vector.tensor_copy(
                    vha[:, sb, :, 0:D],
                    ps.rearrange("p (h d) -> p h d", h=HPC),
                )

            # ---- attention -------------------------------------------------
            # pending: list of deferred output-projection chunks (closures)
            # from the previous q-superblock, drip-fed into this superblock's
            # kb loop so the PE work overlaps the exp stream.
            pending = []

            def emit_outproj(qs):
                chunks = []
                for sb in range(4 * qs, 4 * qs + 4):
                    def go(sb=sb):
                        pso = [
                            ppool.tile([P, QSUP], f32, tag="ps",
                                       name=f"pso_{sb}_{n2}")
                            for n2 in range(2)
                        ]
                        for km in range(2):
                            for n2 in range(2):
                                nc.tensor.matmul(
                                    pso[n2],
                                    ctxT[:, km, sb * P:(sb + 1) * P],
                                    wo_sb[:, km,
                                          n2 * QSUP:(n2 + 1) * QSUP],
                                    start=(km == 0), stop=(km == 1),
                                )
                        for n2 in range(2):
                            ot = osb.tile([P, QSUP], bf16, tag="ot",
                                          name=f"ot_{sb}_{n2}")
                            nc.vector.tensor_copy(out=ot, in_=pso[n2])
                            nc.sync.dma_start(
                                out[sb * P:(sb + 1) * P,
                                    n2 * QSUP:(n2 + 1) * QSUP], ot,
                            )
                    chunks.append(go)
                return chunks

            def qlo_of(kb, qs):
                r = kb - 4 * qs
                return r * P if (causal and r >= 0) else 0

            for qs in range(NSUP):
                for m in range(2):        # head pair (local heads 2m, 2m+1)
                    nkb = 4 * qs + 4 if causal else NKB
                    cps = [
                        cpool.tile([P, QSUP], f32, tag="cps",
                                   name=f"cps_{m}_{qs}_{h2}")
                        for h2 in range(2)
                    ]
                    ats = {}
                    for kb in range(nkb + LAG):
                        if kb < nkb:
                            qlo = qlo_of(kb, qs)
                            qsl = slice(qs * QSUP + qlo, (qs + 1) * QSUP)
                            st = stp.tile([P, 2, QSUP], f32, tag="st2",
                                          name=f"st_{m}_{qs}_{kb}")
                            for h2 in range(2):
                                nc.tensor.matmul(
                                    st[:, h2, qlo:],
                                    khT[:, 2 * m + h2,
                                        kb * P:(kb + 1) * P],
                                    qhT[:, m, qsl],
                                    start=True, stop=True,
                                )
                            at = attn.tile([P, 2, QSUP], bf16, tag="at",
                                           name=f"at_{m}_{qs}_{kb}")
                            nc.scalar.activation(at[:, :, qlo:],
                                                 st[:, :, qlo:], AF.Exp)
                            if causal and kb - 4 * qs >= 0:
                                nc.gpsimd.tensor_mul(
                                    at[:, :, qlo:qlo + P],
                                    at[:, :, qlo:qlo + P],
                                    cm_sb,
                                )
                            ats[kb] = at
                        if pending and m == 0 and 2 <= kb < 2 + len(pending):
                            pending[kb - 2]()
                            if kb - 2 == 3:
                                pending = []
                        j = kb - LAG
                        if 0 <= j < nkb:
                            qlo = qlo_of(j, qs)
                            for h2 in range(2):
                                nc.tensor.matmul(
                                    cps[h2][:, qlo:],
                                    vha[:, j, 2 * m + h2, :],
                                    ats[j][:, h2, qlo:],
                                    start=(j == 0), stop=(j == nkb - 1),
                                )
                            del ats[j]
                    # normalization: both heads' row-sums into one tile,
                    # one reciprocal, two muls into ctxT pair-slots
                    sums = norm.tile([P, QSUP], f32, tag="sums")
                    nc.vector.tensor_copy(out=sums[0:D, :], in_=cps[0][D:, :])
                    nc.vector.tensor_copy(out=sums[D:, :], in_=cps[1][D:, :])
                    rec = norm.tile([P, QSUP], f32, tag="rec")
                    nc.vector.reciprocal_approx_fast(out=rec, in_=sums)
                    qsl = slice(qs * QSUP, (qs + 1) * QSUP)
                    nc.vector.tensor_mul(
                        ctxT[0:D, m, qsl], cps[0][0:D, :], rec[0:D, :],
                    )
                    nc.vector.tensor_mul(
                        ctxT[D:, m, qsl], cps[1][0:D, :], rec[D:, :],
                    )

                for chunk in pending:   # not fully drained (short kb loop)
                    chunk()
                pending = emit_outproj(qs)

            for chunk in pending:
                chunk()

    nc.finalize()
    return nc


def _get_nc(causal: bool):
    key = ("nc", causal)
    if key not in _CACHE:
        _CACHE[key] = _build_nc(causal)
    return _CACHE[key]


def _bf(a):
    return np.ascontiguousarray(a, dtype=np.float32).astype(BF16)


def _wperm(wT, nko):
    """[nko*128, M] -> [128, nko, M] so each SBUF partition's data is one
    contiguous run in DRAM (single DMA descriptor per partition)."""
    wT = np.asarray(wT, np.float32)
    m = wT.shape[1]
    return np.ascontiguousarray(
        wT.reshape(nko, P, m).transpose(1, 0, 2)).astype(BF16)


def kernel(q, k, v, mask, Wq, bq, Wk, bk, Wv, bv, Wo, bo):
    q = np.asarray(q, np.float32)
    k = np.asarray(k, np.float32)
    v = np.asarray(v, np.float32)
    mask = np.asarray(mask)
    Wq, bq = np.asarray(Wq, np.float32), np.asarray(bq, np.float32)
    Wk, bk = np.asarray(Wk, np.float32), np.asarray(bk, np.float32)
    Wv, bv = np.asarray(Wv, np.float32), np.asarray(bv, np.float32)
    Wo, bo = np.asarray(Wo, np.float32), np.asarray(bo, np.float32)

    m2 = mask.reshape(S, S) != 0
    if m2.all():
        causal = False
    else:
        tri = np.tril(np.ones((S, S), bool))
        assert (m2 == tri).all(), "only causal or all-ones masks supported"
        causal = True

    nc = _get_nc(causal)

    cm1 = np.asarray(
        np.arange(P)[:, None] <= np.arange(P)[None, :], np.float32
    ).astype(BF16)  # [k, q] keep-region of the diagonal 128-band
    cm = np.ascontiguousarray(np.stack([cm1, cm1], axis=1))  # [P, 2, P]

    xT = {}
    for b in range(B):
        xT[("q", b)] = _bf(q[b].T)
        xT[("k", b)] = _bf(k[b].T)
        xT[("v", b)] = _bf(v[b].T)

    in_maps = []
    for c in range(NCORES):
        b = c // 4
        rows = slice((c % 4) * DC, (c % 4) * DC + DC)
        bq_s = (bq[rows] / SCALE).reshape(2, P).T
        bk_s = bk[rows].reshape(2, P).T
        in_maps.append({
            "xqT": xT[("q", b)],
            "xkT": xT[("k", b)],
            "xvT": xT[("v", b)],
            "wqT": _wperm(Wq[rows].T / SCALE, 8),
            "wkT": _wperm(Wk[rows].T, 8),
            "wvT": _wperm(Wv[rows].T, 8),
            "woT": _wperm(Wo[:, rows].T, 2),
            "bqk": np.ascontiguousarray(
                np.concatenate([bq_s, bk_s], axis=1), np.float32),
            "cmask": cm,
        })

    res = run_bass_kernel_spmd(nc, in_maps, core_ids=list(range(NCORES)))
    LAST["exec_time_ns"] = res.exec_time_ns
    LAST["results"] = res

    host_bias = (bo + bv @ Wo.T).astype(np.float32)
    out = np.zeros((B, S, E), np.float32)
    for c in range(NCORES):
        out[c // 4] += res.results[c]["out"].astype(np.float32)
    out += host_bias
    return out
